# revision 1
# baseline (speedup 1.0000x reference)
"""Trainium2 Bass kernel for nn_BDH_39127152067244 (dense_transformer).

Sharding: 8 cores = (b, h) pairs — b = core // 4, h = core % 4. Each core
computes its head's share of every layer; the only cross-core communication
is a 4-rank AllReduce of the per-head yMLP partial (replica groups {0..3}
and {4..7}), issued once per t-half per layer.

Structure: each layer is emitted as two serialized t-half streams
(t in [0,256) then [256,512)). The AllReduce for half h is issued right
after that half's decoder matmul finishes and is consumed only at the START
of the next layer's half-h stream — a full opposite-half stream (~60us) is
always in flight between issue and use, so collective latency is hidden and
the PE never idles long enough to drop to the cold HAM clock. A dummy
AllReduce at kernel start absorbs the expensive first-collective path.

Layout tricks (vs the reference):
  - The N axis (8192) is deinterleaved on the host (even n first, odd n
    second). Rope's interleaved pair-swap becomes a half-offset of whole
    128-partition tiles. Both rope tables are then column-periodic with
    period N/2, so only [T, 4096] of cos and sin are stored/streamed; the
    sign of the sin term is folded into add-vs-subtract in the vector op.
  - x_sparse is computed directly in transposed [N, T] layout.
  - scores: the Gram matrix of rope'd activations is symmetric, so the
    strict-lower-triangular masked scores in [t, s] layout equal the
    strict-upper masked Gram in [s, t] layout — computed directly as the
    yKV matmul's lhsT. Only diagonal 128x128 blocks are masked; fully-kept
    blocks are copied and fully-masked blocks never computed.
  - All matmuls run in bf16 with f32 PSUM accumulation; LayerNorms and the
    residual stream stay f32.
"""

import math
import sys
from contextlib import ExitStack

import numpy as np
import ml_dtypes

sys.path.insert(0, "/opt/trn_rl_repo")

import concourse.bass as bass  # noqa: E402
import concourse.bacc as bacc  # noqa: E402
import concourse.mybir as mybir  # noqa: E402
import concourse.tile as tile  # noqa: E402
from concourse.bass import ds  # noqa: E402
from concourse.bass_utils import run_bass_kernel_spmd  # noqa: E402
from concourse.masks import make_identity  # noqa: E402

BF16 = ml_dtypes.bfloat16
BF = mybir.dt.bfloat16
FP32 = mybir.dt.float32
AF = mybir.ActivationFunctionType
ALU = mybir.AluOpType

# Problem constants (hardcoded per the harness contract).
N_LAYER = 6
D = 256
NH = 4
N = 8192
HALF = N // 2
VOCAB = 256
B, T = 2, 512
THETA = 2.0**16
EPS = 1e-5

P = 128          # partitions
NT = N // P      # 64 n-tiles
G4 = 4           # n-tiles per rope group
NG = NT // G4    # 16 groups
VG = 8           # n-tiles per V tile
NVG = NT // VG   # 8 V tiles
TC = T // P      # 4 t-chunks
TH = T // 2      # 256 columns per half
DT = D // P      # 2 d-tiles
N_CORES = 8
RG = [[0, 1, 2, 3], [4, 5, 6, 7]]

# rope pair emission order: (g, g^8) adjacent so the shared table tile is
# used twice back to back; A-phase V-tile order matches.
PAIR_G = [0, 8, 1, 9, 2, 10, 3, 11, 4, 12, 5, 13, 6, 14, 7, 15]
A_VG = [0, 4, 1, 5, 2, 6, 3, 7]

_CACHE: dict = {}


def _build_bass():
    nc = bacc.Bacc("TRN2", num_devices=N_CORES)

    x0_d = nc.dram_tensor("x0", [P, TC, D], FP32, kind="ExternalInput")
    x0bf_d = nc.dram_tensor("x0bf", [P, TC, D], BF, kind="ExternalInput")
    x0T_d = nc.dram_tensor("x0T", [P, DT, T], BF, kind="ExternalInput")
    enc_d = nc.dram_tensor("enc", [P, DT, NT, P], BF, kind="ExternalInput")
    encv_d = nc.dram_tensor("encv", [P, DT, NT, P], BF, kind="ExternalInput")
    dec_d = nc.dram_tensor("dec", [P, NT, D], BF, kind="ExternalInput")
    tab_d = nc.dram_tensor("tabb", [P, 8, 2 * G4, T], BF, kind="ExternalInput")
    mask_d = nc.dram_tensor("maskb", [P, P], BF, kind="ExternalInput")
    lm_d = nc.dram_tensor("lm", [P, DT, VOCAB], BF, kind="ExternalInput")
    out_d = nc.dram_tensor("logits", [P, TC, VOCAB], FP32, kind="ExternalOutput")

    with tile.TileContext(nc) as tc, ExitStack() as ctx:
        sb = ctx.enter_context(tc.tile_pool(name="sb", bufs=1))
        wres = ctx.enter_context(tc.tile_pool(name="wres", bufs=1))
        vpool = ctx.enter_context(tc.tile_pool(name="vpool", bufs=8))
        qr0p = ctx.enter_context(tc.tile_pool(name="qr0p", bufs=NG))
        qr1p = ctx.enter_context(tc.tile_pool(name="qr1p", bufs=4))
        tmpp = ctx.enter_context(tc.tile_pool(name="tmpp", bufs=2))
        tabp = ctx.enter_context(tc.tile_pool(name="tabp", bufs=3))
        evp = ctx.enter_context(tc.tile_pool(name="evp", bufs=2))
        decp = ctx.enter_context(tc.tile_pool(name="decp", bufs=2))
        stp = ctx.enter_context(tc.tile_pool(name="stp", bufs=1))
        yp = ctx.enter_context(tc.tile_pool(name="yp", bufs=2))
        xyp = ctx.enter_context(tc.tile_pool(name="xyp", bufs=2))
        xp = ctx.enter_context(tc.tile_pool(name="xp", bufs=2))
        statp = ctx.enter_context(tc.tile_pool(name="statp", bufs=4))
        aqp = ctx.enter_context(tc.tile_pool(name="aqp", bufs=2, space="PSUM"))
        mmp = ctx.enter_context(tc.tile_pool(name="mmp", bufs=2, space="PSUM"))
        drm = ctx.enter_context(tc.tile_pool(name="drm", bufs=2, space="DRAM"))

        ident = sb.tile([P, P], BF, name="ident")
        make_identity(nc, ident)
        epst = sb.tile([P, 1], FP32, name="epst")
        nc.vector.memset(epst, EPS)
        maskd = sb.tile([P, P], BF, name="maskd")
        nc.sync.dma_start(out=maskd, in_=mask_d[:])
        lmt = sb.tile([P, DT, VOCAB], BF, name="lmt")
        nc.sync.dma_start(out=lmt, in_=lm_d[:])

        enc_sb = wres.tile([P, DT, NT, P], BF, name="enc_sb")

        # state carried between phases: per half h -> tile
        ST = {"ymr": {}, "xf": {}, "xbf": {}, "xT": {}}

        def rstd_of(var_ap, name):
            r = statp.tile([P, 1], FP32, tag="rs", name=f"rs_{name}")
            nc.scalar.activation(out=r, in_=var_ap, func=AF.Sqrt, bias=epst)
            nc.vector.reciprocal(r, r)
            return r

        def ln_stats(src_ap, name):
            s6 = statp.tile([P, 6], FP32, tag="bst", name=f"st_{name}")
            nc.vector.bn_stats(out=s6, in_=src_ap)
            mv = statp.tile([P, 2], FP32, tag="bmv", name=f"mv_{name}")
            nc.vector.bn_aggr(out=mv, in_=s6)
            return mv

        def emit_x_from_dram(h):
            xf = xp.tile([P, 2, D], FP32, tag=f"xf{h}", name=f"xf0_{h}")
            nc.sync.dma_start(out=xf, in_=x0_d[:, ds(h * 2, 2), :])
            xbf = xp.tile([P, 2, D], BF, tag=f"xbf{h}", name=f"xbf0_{h}")
            nc.sync.dma_start(out=xbf, in_=x0bf_d[:, ds(h * 2, 2), :])
            xT = xp.tile([P, DT, TH], BF, tag=f"xT{h}", name=f"xT0_{h}")
            nc.sync.dma_start(out=xT, in_=x0T_d[:, :, ds(h * TH, TH)])
            ST["xf"][h], ST["xbf"][h], ST["xT"][h] = xf, xbf, xT

        def emit_x_update(l, h):
            """x_new = LN(x_old + LN(ymr)); consumes AR(l-1, h)."""
            ymr = ST["ymr"][h]
            xf_old = ST["xf"][h]
            xf = xp.tile([P, 2, D], FP32, tag=f"xf{h}", name=f"xf{l}_{h}")
            xbf = xp.tile([P, 2, D], BF, tag=f"xbf{h}", name=f"xbf{l}_{h}")
            xmid = xp.tile([P, 2, D], FP32, tag="xmid", name=f"xm{l}_{h}", bufs=1)
            for jl in range(2):
                mv1 = ln_stats(ymr[:, jl, :], f"y{l}_{h}_{jl}")
                r1 = rstd_of(mv1[:, 1:2], f"y{l}_{h}_{jl}")
                # mean-shift of LN(ymr) is absorbed by the outer LN
                nc.vector.scalar_tensor_tensor(
                    out=xmid[:, jl, :],
                    in0=ymr[:, jl, :],
                    scalar=r1,
                    in1=xf_old[:, jl, :],
                    op0=ALU.mult,
                    op1=ALU.add,
                )
                mv2 = ln_stats(xmid[:, jl, :], f"x{l}_{h}_{jl}")
                r2 = rstd_of(mv2[:, 1:2], f"x{l}_{h}_{jl}")
                nc.vector.tensor_scalar(
                    out=xf[:, jl, :],
                    in0=xmid[:, jl, :],
                    scalar1=mv2[:, 0:1],
                    scalar2=r2,
                    op0=ALU.subtract,
                    op1=ALU.mult,
                )
                nc.scalar.copy(out=xbf[:, jl, :], in_=xf[:, jl, :])
            xT = xp.tile([P, DT, TH], BF, tag=f"xT{h}", name=f"xT{l}_{h}")
            tp4x = mmp.tile([P, 2, DT, P], BF, tag="mm", name=f"xtp{l}_{h}")
            for jl in range(2):
                for dt_ in range(DT):
                    nc.tensor.transpose(
                        tp4x[:, jl, dt_, :], xbf[:, jl, ds(dt_ * P, P)], ident
                    )
            for dt_ in range(DT):
                nc.scalar.copy(
                    out=xT[:, dt_, :].rearrange("p (a b) -> p a b", a=2),
                    in_=tp4x[:, :, dt_, :],
                )
            ST["xf"][h], ST["xbf"][h], ST["xT"][h] = xf, xbf, xT

        def emit_half(l, h):
            xT = ST["xT"][h]

            # ---------- A: V = relu(enc^T @ x^T) on this half's columns ----
            V = {}
            for vgi, vg in enumerate(A_VG):
                if l == 0 and h == 0 and vgi >= 2 and vgi % 2 == 0:
                    for w in range(6):
                        wt0 = aqp.tile(
                            [P, 4, TH], BF, tag="aq", name=f"aw{vgi}_{w}"
                        )
                        nc.tensor.transpose(wt0[:, 0, 0:P], ident, ident)
                vt = vpool.tile([P, VG, TH], BF, tag="v", name=f"v{l}_{h}_{vg}")
                V[vg] = vt
                for q in range(2):
                    ps = aqp.tile([P, 4, TH], FP32, tag="aq", name=f"aps{l}_{h}_{vg}_{q}")
                    for j in range(4):
                        # enc storage order is A_VG order (host-reordered)
                        nt_ = vgi * VG + q * 4 + j
                        for dt_ in range(DT):
                            nc.tensor.matmul(
                                ps[:, j, :],
                                lhsT=enc_sb[:, dt_, nt_, :],
                                rhs=xT[:, dt_, :],
                                start=(dt_ == 0),
                                stop=(dt_ == DT - 1),
                            )
                    nc.scalar.activation(
                        out=vt[:, ds(q * 4, 4), :], in_=ps, func=AF.Relu
                    )

            # ---------- rope + Gram (C), interleaved per pair-group --------
            if h == 0:
                gps = mmp.tile([P, 2, TH], FP32, tag="mm", name=f"gps{l}_0")
            else:
                gps = mmp.tile([P, 4, TH], FP32, tag="mm", name=f"gps{l}_1")
            QR = {}
            for pi, g in enumerate(PAIR_G):
                gm = g % 8
                if g < 8:
                    tabg = tabp.tile(
                        [P, 2 * G4, TH], BF, tag="tab", name=f"tb{l}_{h}_{gm}"
                    )
                    nc.sync.dma_start(
                        out=tabg, in_=tab_d[:, gm, :, ds(h * TH, TH)]
                    )
                    cosg, sing = tabg[:, 0:G4, :], tabg[:, G4:, :]
                pool = qr0p if h == 0 else qr1p
                qr = pool.tile([P, G4, TH], BF, tag=f"q{h}", name=f"qr{l}_{h}_{g}")
                QR[g] = qr
                vg_, off = divmod(g * G4, VG)
                pvg_, poff = divmod((g ^ (NG // 2)) * G4, VG)
                p2 = tmpp.tile([P, G4, TH], BF, tag="p2", name=f"p2_{l}_{h}_{g}")
                nc.vector.tensor_mul(qr, V[vg_][:, ds(off, G4), :], cosg)
                nc.vector.tensor_mul(p2, V[pvg_][:, ds(poff, G4), :], sing)
                if g < 8:
                    nc.vector.tensor_sub(qr, qr, p2)
                else:
                    nc.vector.tensor_add(qr, qr, p2)

                # Gram accumulation for this k-group
                # NOTE: start=True clears has_written bits for the WHOLE psum
                # bank, so only the first matmul touching each bank may set it;
                # sibling regions in the same bank start with start=False and
                # rely on that clear (their first write then overwrites).
                # gps h0 [P,2,TH] = 1 bank; gps h1 [P,4,TH] = 2 banks (j01/j23).
                ki0 = pi * G4
                for i in range(G4):
                    ki = ki0 + i
                    stp_ = ki == NT - 1
                    if h == 0:
                        # rows j=0 (cols 0:256), j=1 (cols 128:256)
                        nc.tensor.matmul(
                            gps[:, 0, :],
                            lhsT=qr[:, i, 0:P],
                            rhs=qr[:, i, :],
                            start=(ki == 0),
                            stop=stp_,
                        )
                        nc.tensor.matmul(
                            gps[:, 1, P:TH],
                            lhsT=qr[:, i, P:TH],
                            rhs=qr[:, i, P:TH],
                            start=False,
                            stop=stp_,
                        )
                    else:
                        qr0 = _QR0[g]
                        for j in range(4):
                            lhs = (
                                qr0[:, i, ds((j % 2) * P, P)]
                                if j < 2
                                else qr[:, i, ds((j - 2) * P, P)]
                            )
                            st_ = (ki == 0) and (j % 2 == 0)
                            if j < 3:
                                nc.tensor.matmul(
                                    gps[:, j, :],
                                    lhsT=lhs,
                                    rhs=qr[:, i, :],
                                    start=st_,
                                    stop=stp_,
                                )
                            else:
                                nc.tensor.matmul(
                                    gps[:, 3, P:TH],
                                    lhsT=lhs,
                                    rhs=qr[:, i, P:TH],
                                    start=st_,
                                    stop=stp_,
                                )
            if h == 0:
                _QR0.clear()
                _QR0.update(QR)

            # ---------- masked score extraction ----------------------------
            if h == 0:
                st0 = stp.tile([P, 2, TH], BF, tag="st0", name=f"st0_{l}")
                nc.vector.tensor_mul(st0[:, 0, 0:P], gps[:, 0, 0:P], maskd)
                nc.vector.tensor_mul(st0[:, 1, P:TH], gps[:, 1, P:TH], maskd)
                nc.scalar.copy(out=st0[:, 0, P:TH], in_=gps[:, 0, P:TH])
                stt = st0
            else:
                st1 = stp.tile([P, 4, TH], BF, tag="st1", name=f"st1_{l}")
                nc.scalar.copy(out=st1[:, 0:2, :], in_=gps[:, 0:2, :])
                nc.vector.tensor_mul(st1[:, 2, 0:P], gps[:, 2, 0:P], maskd)
                nc.vector.tensor_mul(st1[:, 3, P:TH], gps[:, 3, P:TH], maskd)
                nc.scalar.copy(out=st1[:, 2, P:TH], in_=gps[:, 2, P:TH])
                stt = st1

            # ---------- D: yKV rows of this half, then LN ------------------
            dps = mmp.tile([P, 2, D], FP32, tag="mm", name=f"dps{l}_{h}")
            xbf0 = ST["xbf"][0]
            xbf1 = ST["xbf"].get(1)
            yln = yp.tile([P, 2, D], BF, tag="yln", name=f"yln{l}_{h}")
            ylnT = yp.tile([P, DT, TH], BF, tag="ylnT", name=f"ylnT{l}_{h}")
            tp4 = mmp.tile([P, 2, DT, P], BF, tag="mm", name=f"ytp{l}_{h}")
            for jl in range(2):
                jp = h * 2 + jl
                for i in range(jp + 1):
                    if h == 0:
                        lhs = stt[:, i, ds(jp * P, P)]
                    else:
                        lhs = stt[:, i, ds(jl * P, P)]
                    rhs = xbf0[:, i, :] if i < 2 else xbf1[:, i - 2, :]
                    # dps is one bank: only (jl==0, i==0) may set start
                    nc.tensor.matmul(
                        dps[:, jl, :],
                        lhsT=lhs,
                        rhs=rhs,
                        start=(jl == 0 and i == 0),
                        stop=(i == jp),
                    )
                # LN of chunk jl overlaps chunk jl+1's matmuls (vector side)
                mv = ln_stats(dps[:, jl, :], f"d{l}_{h}_{jl}")
                r = rstd_of(mv[:, 1:2], f"d{l}_{h}_{jl}")
                nc.vector.tensor_scalar(
                    out=yln[:, jl, :],
                    in0=dps[:, jl, :],
                    scalar1=mv[:, 0:1],
                    scalar2=r,
                    op0=ALU.subtract,
                    op1=ALU.mult,
                )
            # transposes AFTER both D chunks so they never head-block D's
            # matmuls in the in-order tensor queue
            for jl in range(2):
                for dt_ in range(DT):
                    nc.tensor.transpose(
                        tp4[:, jl, dt_, :], yln[:, jl, ds(dt_ * P, P)], ident
                    )
            for dt_ in range(DT):
                nc.scalar.copy(
                    out=ylnT[:, dt_, :].rearrange("p (a b) -> p a b", a=2),
                    in_=tp4[:, :, dt_, :],
                )

            # ---------- E (gated y_sparse) + F (decoder), fused per group --
            fps = mmp.tile([P, 2, D], FP32, tag="mm", name=f"fps{l}_{h}")
            for g in range(NG):
                if g % 4 == 0:
                    # bulk streams ride the scalar HWDGE ring so they never
                    # queue ahead of the latency-critical table loads (sync)
                    decq = decp.tile([P, 16, D], BF, tag="dec", name=f"dq{l}_{h}_{g}")
                    nc.scalar.dma_start(out=decq, in_=dec_d[:, ds(g * G4, 16), :])
                    evq = evp.tile(
                        [P, DT, 16, P], BF, tag="ev", name=f"ev{l}_{h}_{g}"
                    )
                    nc.scalar.dma_start(
                        out=evq, in_=encv_d[:, :, ds(g * G4, 16), :]
                    )
                ps = aqp.tile([P, 4, TH], FP32, tag="aq", name=f"eps{l}_{h}_{g}")
                for j in range(G4):
                    for dt_ in range(DT):
                        nc.tensor.matmul(
                            ps[:, j, :],
                            lhsT=evq[:, dt_, (g % 4) * G4 + j, :],
                            rhs=ylnT[:, dt_, :],
                            start=(dt_ == 0),
                            stop=(dt_ == DT - 1),
                        )
                ys = yp.tile([P, G4, TH], BF, tag="ys", name=f"ys{l}_{h}_{g}")
                nc.scalar.activation(out=ys, in_=ps, func=AF.Relu)
                xy = xyp.tile([P, G4, TH], BF, tag="xy", name=f"xy{l}_{h}_{g}")
                nc.vector.tensor_mul(
                    xy, ys, V[g // 2][:, ds((g % 2) * G4, G4), :]
                )
                for i in range(G4):
                    k = g * G4 + i
                    for m in range(2):
                        # fps is one bank: only (k==0, m==0) may set start
                        nc.tensor.matmul(
                            fps[:, m, :],
                            lhsT=xy[:, i, ds(m * P, P)],
                            rhs=decq[:, k % 16, :],
                            start=(k == 0 and m == 0),
                            stop=(k == NT - 1),
                        )

            # ---------- AllReduce of this half's yMLP partial --------------
            ymlp = yp.tile([P, 2, D], BF, tag="ym", name=f"ym{l}_{h}")
            nc.scalar.copy(out=ymlp, in_=fps)
            cc_in = drm.tile([P, 2, D], BF, tag=f"ci{h}", name=f"ci{l}_{h}")
            cc_out = drm.tile([P, 2, D], BF, tag=f"co{h}", name=f"co{l}_{h}")
            nc.gpsimd.dma_start(out=cc_in[:], in_=ymlp)
            nc.gpsimd.collective_compute(
                "AllReduce", ALU.add, replica_groups=RG,
                ins=[cc_in[:]], outs=[cc_out[:]],
            )
            ymr = yp.tile([P, 2, D], BF, tag=f"ymr{h}", name=f"ymr{l}_{h}")
            nc.sync.dma_start(out=ymr, in_=cc_out[:])
            ST["ymr"][h] = ymr

        _QR0: dict = {}

        # startup: x0 first (small, unblocks A), then warmup AR, then enc
        # in V-tile chunks ordered as A consumes them
        for h in range(2):
            emit_x_from_dram(h)
        zt = sb.tile([P, 2, D], BF, name="zt")
        nc.vector.memset(zt, 0.0)
        w_in = drm.tile([P, 2, D], BF, tag="wi", name="w_in")
        w_out = drm.tile([P, 2, D], BF, tag="wo", name="w_out")
        nc.gpsimd.dma_start(out=w_in[:], in_=zt)
        nc.gpsimd.collective_compute(
            "AllReduce", ALU.add, replica_groups=RG, ins=[w_in[:]], outs=[w_out[:]]
        )
        nc.gpsimd.dma_start(out=zt, in_=w_out[:])  # sink back into zt
        # enc is host-reordered so storage order == A's consumption order;
        # two bulk loads on the scalar ring
        for ch in range(4):
            nc.scalar.dma_start(
                out=enc_sb[:, :, ds(ch * 16, 16), :],
                in_=enc_d[:, :, ds(ch * 16, 16), :],
            )
        # PE warmup against DMA jitter at the very start
        for w in range(8):
            wtp = aqp.tile([P, 4, TH], BF, tag="aq", name=f"wtp{w}")
            nc.tensor.transpose(wtp[:, 0, 0:P], ident, ident)

        for l in range(N_LAYER):
            for h in range(2):
                if l > 0:
                    emit_x_update(l, h)
                emit_half(l, h)

        # ---------- final x update + lm head ------------------------------
        for h in range(2):
            emit_x_update(N_LAYER, h)
            xT = ST["xT"][h]
            lps = mmp.tile([P, 2, VOCAB], FP32, tag="mm", name=f"lps{h}")
            for jl in range(2):
                for dt_ in range(DT):
                    # lps is one bank: only (jl==0, dt==0) may set start
                    nc.tensor.matmul(
                        lps[:, jl, :],
                        lhsT=xT[:, dt_, ds(jl * P, P)],
                        rhs=lmt[:, dt_, :],
                        start=(jl == 0 and dt_ == 0),
                        stop=(dt_ == DT - 1),
                    )
            lout = yp.tile([P, 2, VOCAB], FP32, tag="lout", name=f"lout{h}")
            nc.scalar.copy(out=lout, in_=lps)
            nc.sync.dma_start(out=out_d[:, ds(h * 2, 2), :], in_=lout)

    if not nc.is_finalized():
        nc.finalize()
    return nc


def _ln_np(x):
    m = x.mean(-1, keepdims=True)
    v = ((x - m) ** 2).mean(-1, keepdims=True)
    return (x - m) / np.sqrt(v + EPS)


def _make_tables():
    t = np.arange(N, dtype=np.float32)
    q = np.floor(t / 2.0) * 2.0
    freqs = (1.0 / (THETA ** (q / N)) / (2.0 * np.float32(math.pi))).astype(
        np.float32
    )
    phases = np.arange(T, dtype=np.float32)[:, None] * freqs[None, :]
    ph = np.float32(np.float32(phases % 1.0) * np.float32(2.0 * math.pi))
    return np.cos(ph).astype(np.float32), np.sin(ph).astype(np.float32)


def _prep_inputs(idx, embed_w, encoder, encoder_v, decoder, lm_head):
    perm = np.concatenate([np.arange(HALF) * 2, np.arange(HALF) * 2 + 1])

    cos, sin = _make_tables()
    # both tables are column-periodic with period HALF after deinterleave
    cos_half = cos[:, perm[:HALF]]  # (T, HALF)
    sin_half = sin[:, perm[:HALF]]  # positive table; sign folded into ops
    cos_h = np.ascontiguousarray(
        cos_half.T.reshape(NT // 2, P, T).transpose(1, 0, 2)
    ).astype(BF16)  # [P, 32, T]
    sin_h = np.ascontiguousarray(
        sin_half.T.reshape(NT // 2, P, T).transpose(1, 0, 2)
    ).astype(BF16)
    # pack cos+sin per rope group so each (group, half) is ONE dma:
    # tab[:, gm, 0:4, :] = cos rows, tab[:, gm, 4:8, :] = sin rows
    tab_h = np.empty((P, 8, 2 * G4, T), BF16)
    for gm in range(8):
        tab_h[:, gm, 0:G4, :] = cos_h[:, gm * G4 : (gm + 1) * G4, :]
        tab_h[:, gm, G4:, :] = sin_h[:, gm * G4 : (gm + 1) * G4, :]

    # strict-upper 128x128 block mask: keep (p, c) when c > p
    mask_h = (np.arange(P)[None, :] > np.arange(P)[:, None]).astype(BF16)

    lm_h = np.ascontiguousarray(
        lm_head.reshape(DT, P, VOCAB).transpose(1, 0, 2)
    ).astype(BF16)

    x0 = _ln_np(embed_w[idx].astype(np.float32))  # (B, T, D)

    dec3 = decoder.reshape(NH, N, D)

    per_core = []
    for core in range(N_CORES):
        b, h = divmod(core, NH)
        enc_p = encoder[h][:, perm]  # (D, N)
        encv_p = encoder_v[h][:, perm]
        dec_p = dec3[h][perm, :]  # (N, D)

        enc_h = np.ascontiguousarray(
            enc_p.reshape(DT, P, NT, P).transpose(1, 0, 2, 3)
        ).astype(BF16)
        # reorder n-tiles so storage order matches A's A_VG consumption order
        a_order = np.concatenate(
            [np.arange(vg * VG, (vg + 1) * VG) for vg in A_VG]
        )
        enc_h = np.ascontiguousarray(enc_h[:, :, a_order, :])
        encv_h = np.ascontiguousarray(
            encv_p.reshape(DT, P, NT, P).transpose(1, 0, 2, 3)
        ).astype(BF16)
        dec_h = np.ascontiguousarray(
            dec_p.reshape(NT, P, D).transpose(1, 0, 2)
        ).astype(BF16)

        xb = x0[b]  # (T, D) f32
        x0_c = np.ascontiguousarray(
            xb.reshape(TC, P, D).transpose(1, 0, 2)
        ).astype(np.float32)
        x0bf_c = x0_c.astype(BF16)
        x0T_c = np.ascontiguousarray(
            xb.T.reshape(DT, P, T).transpose(1, 0, 2)
        ).astype(BF16)

        per_core.append(
            {
                "x0": x0_c,
                "x0bf": x0bf_c,
                "x0T": x0T_c,
                "enc": enc_h,
                "encv": encv_h,
                "dec": dec_h,
                "tabb": tab_h,
                "maskb": mask_h,
                "lm": lm_h,
            }
        )
    return per_core


def _get_nc():
    if "nc" not in _CACHE:
        _CACHE["nc"] = _build_bass()
    return _CACHE["nc"]


def kernel(idx, embed_w, encoder, encoder_v, decoder, lm_head, **extra):
    idx = np.asarray(idx)
    embed_w = np.asarray(embed_w, dtype=np.float32)
    encoder = np.asarray(encoder, dtype=np.float32)
    encoder_v = np.asarray(encoder_v, dtype=np.float32)
    decoder = np.asarray(decoder, dtype=np.float32)
    lm_head = np.asarray(lm_head, dtype=np.float32)

    nc = _get_nc()
    in_maps = _prep_inputs(idx, embed_w, encoder, encoder_v, decoder, lm_head)
    res = run_bass_kernel_spmd(nc, in_maps, core_ids=list(range(N_CORES)))
    _CACHE["last_results"] = res

    out = np.zeros((B, T, VOCAB), np.float32)
    for b in range(B):
        lg = res.results[b * NH]["logits"]  # [P, TC, VOCAB]
        out[b] = lg.transpose(1, 0, 2).reshape(T, VOCAB)
    return out


if __name__ == "__main__":
    rng = np.random.default_rng(0)
    ins = {
        "idx": rng.integers(0, VOCAB, (B, T)).astype(np.int32),
        "embed_w": (0.02 * rng.standard_normal((VOCAB, D))).astype(np.float32),
        "encoder": (0.02 * rng.standard_normal((NH, D, N))).astype(np.float32),
        "encoder_v": (0.02 * rng.standard_normal((NH, D, N))).astype(np.float32),
        "decoder": (0.02 * rng.standard_normal((NH * N, D))).astype(np.float32),
        "lm_head": (0.02 * rng.standard_normal((D, VOCAB))).astype(np.float32),
    }
    out = kernel(**ins)
    print("out", out.shape, out.dtype, float(np.abs(out).max()))

